# revision 1
# baseline (speedup 1.0000x reference)
"""Batch triplet loss on 8 TRN2 NeuronCores — symmetric half-Gram, v3.

Same math/coverage as v2 (see kernel_v2.py docstring), restructured:
  - one resident SBUF tile `movs_all` [128, KT*MOVW]; one batched DMA per slab
  - main matmuls grouped over 4 slabs per (it, k) so the stationary weight
    load amortizes over 4 matmuls (PSUM group of 4 banks)
  - direction-2: running elementwise max of F_it across the 8 i-tiles
    (F[p,j] = max over i = p mod 128), then 4 fp16 DMA-transposes + one
    3D reduce per d2 slab (24 transposes/rep instead of 192)
"""

import os
from contextlib import ExitStack

import ml_dtypes
import numpy as np

import concourse.bass as bass
import concourse.tile as tile
from concourse import bacc, bass_utils, mybir

N = 8192
D = 1024
NCORES = 8
OWN = N // NCORES       # 1024
KT = D // 128           # 8
JW = 512
NPAN = 5
MOVW = NPAN * OWN       # 5120
NSLAB = MOVW // JW      # 10
IT = OWN // 128         # 8
ND2 = 3
EPS = 1e-6
MARGIN = 0.5

BF16 = mybir.dt.bfloat16
F16 = mybir.dt.float16
F32 = mybir.dt.float32

_NC = None

# slab groups for the main matmul loop: own panel first (2 banks), then 4+4
SLAB_GROUPS = [[0, 1], [2, 3, 4, 5], [6, 7, 8, 9]]


def _build_nc():
    REPEAT = int(os.environ.get("KBENCH_REPEAT", "1"))
    nc = bacc.Bacc("TRN2", target_bir_lowering=False, debug=False)
    mov = nc.dram_tensor("mov", [D, MOVW], BF16, kind="ExternalInput").ap()
    posT = nc.dram_tensor("posT", [128, KT], BF16, kind="ExternalInput").ap()
    out_m1 = nc.dram_tensor("out_m1", [128, IT], F32, kind="ExternalOutput").ap()
    out_m2 = nc.dram_tensor("out_m2", [128, ND2 * IT], F32, kind="ExternalOutput").ap()
    out_dot = nc.dram_tensor("out_dot", [128, IT], F32, kind="ExternalOutput").ap()
    out_sq = nc.dram_tensor("out_sq", [1, OWN], F32, kind="ExternalOutput").ap()

    mov_v = mov.rearrange("(k p) w -> p k w", p=128)  # [128, KT, MOVW]

    with ExitStack() as ctx:
        tc = ctx.enter_context(tile.TileContext(nc))
        const = ctx.enter_context(tc.tile_pool(name="const", bufs=1))
        big = ctx.enter_context(tc.tile_pool(name="big", bufs=1))
        sqp = ctx.enter_context(tc.tile_pool(name="sqp", bufs=2))
        repp = ctx.enter_context(tc.tile_pool(name="repp", bufs=6))
        ssqp = ctx.enter_context(tc.tile_pool(name="ssqp", bufs=3))
        ttp = ctx.enter_context(tc.tile_pool(name="ttp", bufs=3))
        f16p = ctx.enter_context(tc.tile_pool(name="f16p", bufs=3))
        facp = ctx.enter_context(tc.tile_pool(name="facp", bufs=5))
        ftp = ctx.enter_context(tc.tile_pool(name="ftp", bufs=2))
        resp = ctx.enter_context(tc.tile_pool(name="resp", bufs=1))
        ps_mm = ctx.enter_context(tc.tile_pool(name="ps_mm", bufs=4, space="PSUM"))
        ps_sq = ctx.enter_context(tc.tile_pool(name="ps_sq", bufs=2, space="PSUM"))
        ps_rep = ctx.enter_context(tc.tile_pool(name="ps_rep", bufs=1, space="PSUM"))
        ps_sm = ctx.enter_context(tc.tile_pool(name="ps_sm", bufs=1, space="PSUM"))

        onesK = const.tile([128, 1], BF16, tag="onesK", name="onesK")
        nc.vector.memset(onesK[:], 1.0)
        ones1 = const.tile([1, 128], F32, tag="ones1", name="ones1")
        nc.vector.memset(ones1[:], 1.0)
        post = const.tile([128, KT], BF16, tag="post", name="post")
        nc.gpsimd.dma_start(post[:], posT[:])

        movs_all = big.tile([128, KT * MOVW], BF16, tag="mv", name="movs_all")
        mv = movs_all[:].rearrange("p (k w) -> p k w", k=KT)  # [128, KT, MOVW]

        def mslice(k, c0, w):
            return movs_all[:, k * MOVW + c0 : k * MOVW + c0 + w]

        sqi = resp.tile([128, IT], F32, tag="sqi", name="sqi")
        m1cols = resp.tile([128, IT * NSLAB], F32, tag="m1cols", name="m1cols")
        m2sb = resp.tile([128, ND2 * IT], F32, tag="m2sb", name="m2sb")
        msb = resp.tile([128, IT], F32, tag="msb", name="msb")
        dotsb = resp.tile([128, IT], F32, tag="dotsb", name="dotsb")

        for rep_i in range(REPEAT):
            # ---- batched DMA, squares, sq_j, rep, sqi per slab ----
            def load_slab(s):
                nc.gpsimd.dma_start(
                    mv[:, :, s * JW : (s + 1) * JW],
                    mov_v[:, :, s * JW : (s + 1) * JW],
                )

            def sq_stage(s):
                own_slab = s < 2
                pssq = ps_sq.tile([1, JW], F32, tag="pssq", name=f"pssq{rep_i}_{s}")
                psqi = (
                    ps_sm.tile([128, 4], F32, tag="small", name=f"psqi{rep_i}_{s}")
                    if own_slab
                    else None
                )
                for k in range(KT):
                    sarr = ssqp.tile([128, JW], BF16, tag="ssq", name=f"ssq{rep_i}_{s}_{k}")
                    nc.vector.tensor_mul(
                        sarr[:], mslice(k, s * JW, JW), mslice(k, s * JW, JW)
                    )
                    nc.tensor.matmul(
                        pssq[:], onesK[:], sarr[:], start=(k == 0), stop=(k == KT - 1)
                    )
                    if own_slab:
                        for c4 in range(4):
                            nc.tensor.matmul(
                                psqi[:, c4 : c4 + 1],
                                sarr[:, bass.ts(c4, 128)],
                                onesK[:],
                                start=(k == 0 and c4 == 0),
                                stop=(k == KT - 1 and c4 == 3),
                            )
                sqs = sqp.tile([1, JW], F32, tag="sqs", name=f"sqs{rep_i}_{s}")
                nc.scalar.copy(sqs[:], pssq[:])
                if own_slab:
                    nc.scalar.copy(sqi[:, s * 4 : (s + 1) * 4], psqi[:])
                    nc.gpsimd.dma_start(out_sq[:, s * JW : (s + 1) * JW], sqs[:])
                psr = ps_rep.tile([128, JW], F32, tag="psr", name=f"psr{rep_i}_{s}")
                nc.tensor.matmul(psr[:], ones1[:], sqs[:], start=True, stop=True)
                rept = repp.tile([128, JW], F32, tag="rep", name=f"rep{rep_i}_{s}")
                nc.scalar.copy(rept[:], psr[:])
                return rept

            reps = {}
            for s in range(2):
                load_slab(s)
            for s in range(2):
                reps[s] = sq_stage(s)

            for gi, grp in enumerate(SLAB_GROUPS):
                if gi > 0:
                    for s in grp:
                        load_slab(s)
                    for s in grp:
                        reps[s] = sq_stage(s)
                # direction-2 running max accumulators
                facc = {}
                for s in grp:
                    if 2 <= s < 2 + 2 * ND2:
                        f = facp.tile([128, JW], F16, tag="facc", name=f"facc{rep_i}_{s}")
                        nc.vector.memset(f[:], -60000.0)
                        facc[s] = f
                # main matmuls: stationary reused across the slabs of the group
                psds = {}
                for it in range(IT):
                    for s in grp:
                        psds[s] = ps_mm.tile(
                            [128, JW], F32, tag="psd", name=f"psd{rep_i}_{gi}_{it}_{s}"
                        )
                    for k in range(KT):
                        for s in grp:
                            nc.tensor.matmul(
                                psds[s][:],
                                mslice(k, it * 128, 128),
                                mslice(k, s * JW, JW),
                                start=(k == 0),
                                stop=(k == KT - 1),
                            )
                    for s in grp:
                        psd = psds[s]
                        to = ttp.tile([128, JW], F16, tag="tt", name=f"tt{rep_i}_{gi}_{it}_{s}")
                        nc.vector.scalar_tensor_tensor(
                            out=to[:],
                            in0=psd[:],
                            scalar=-2.0,
                            in1=reps[s][:],
                            op0=mybir.AluOpType.mult,
                            op1=mybir.AluOpType.add,
                        )
                        nc.vector.reduce_max(
                            m1cols[:, it * NSLAB + s : it * NSLAB + s + 1],
                            to[:],
                            axis=mybir.AxisListType.X,
                            op=mybir.AluOpType.max,
                        )
                        if s in facc:
                            f16t = f16p.tile(
                                [128, JW], F16, tag="f16", name=f"f16{rep_i}_{gi}_{it}_{s}"
                            )
                            nc.scalar.activation(
                                f16t[:],
                                psd[:],
                                mybir.ActivationFunctionType.Identity,
                                bias=sqi[:, it : it + 1],
                                scale=-2.0,
                            )
                            nc.vector.tensor_tensor(
                                facc[s][:], facc[s][:], f16t[:], op=mybir.AluOpType.max
                            )
                # direction-2 tail per d2 slab of this group
                for s in grp:
                    if s not in facc:
                        continue
                    ft = ftp.tile([128, JW], F16, tag="ft", name=f"ft{rep_i}_{s}")
                    for q in range(4):
                        nc.sync.dma_start_transpose(
                            ft[:, bass.ts(q, 128)], facc[s][:, bass.ts(q, 128)]
                        )
                    dd = (s - 2) // 2
                    r = (s - 2) % 2
                    col = dd * IT + r * 4
                    nc.vector.reduce_max(
                        m2sb[:, col : col + 4],
                        ft[:].rearrange("p (q i) -> p q i", i=128),
                        axis=mybir.AxisListType.X,
                        op=mybir.AluOpType.max,
                    )

            for it in range(IT):
                nc.vector.reduce_max(
                    msb[:, it : it + 1],
                    m1cols[:, it * NSLAB : (it + 1) * NSLAB],
                    axis=mybir.AxisListType.X,
                    op=mybir.AluOpType.max,
                )
                psdt = ps_sm.tile([128, 4], F32, tag="small", name=f"psdt{rep_i}_{it}")
                for k in range(KT):
                    nc.tensor.matmul(
                        psdt[:, 0:1],
                        mslice(k, it * 128, 128),
                        post[:, k : k + 1],
                        start=(k == 0),
                        stop=(k == KT - 1),
                    )
                nc.scalar.copy(dotsb[:, it : it + 1], psdt[:, 0:1])

        nc.gpsimd.dma_start(out_m1[:], msb[:])
        nc.gpsimd.dma_start(out_m2[:], m2sb[:])
        nc.gpsimd.dma_start(out_dot[:], dotsb[:])

    nc.compile()
    return nc


def _get_nc():
    global _NC
    if _NC is None:
        _NC = _build_nc()
    return _NC


def _make_in_maps(batch, positive):
    bT16 = np.ascontiguousarray(batch.T).astype(ml_dtypes.bfloat16)
    posT16 = np.ascontiguousarray(
        positive.reshape(KT, 128).T.astype(ml_dtypes.bfloat16)
    )
    in_maps = []
    for c in range(NCORES):
        panels = [
            bT16[:, ((c + d) % NCORES) * OWN : ((c + d) % NCORES + 1) * OWN]
            for d in range(NPAN)
        ]
        in_maps.append(
            {"mov": np.ascontiguousarray(np.concatenate(panels, axis=1)), "posT": posT16}
        )
    return in_maps


def _combine(results, positive):
    f32 = np.float32
    m = np.concatenate(
        [results[c]["out_m1"].T.reshape(-1) for c in range(NCORES)]
    ).astype(f32)
    for d in range(1, ND2 + 1):
        cand = np.empty(N, np.float32)
        for c in range(NCORES):
            tgt = (c + d) % NCORES
            cand[tgt * OWN : (tgt + 1) * OWN] = (
                results[c]["out_m2"][:, (d - 1) * IT : d * IT].T.reshape(-1)
            )
        m = np.maximum(m, cand)
    dot = np.concatenate(
        [results[c]["out_dot"].T.reshape(-1) for c in range(NCORES)]
    ).astype(f32)
    sq = np.concatenate(
        [results[c]["out_sq"].reshape(-1) for c in range(NCORES)]
    ).astype(f32)

    deps2 = f32(D * EPS * EPS)
    pp = f32(np.sum(positive.astype(f32) ** 2, dtype=f32))
    d2max = sq + m + deps2
    max_neg = np.sqrt(np.maximum(d2max, f32(0.0)))
    pos2 = sq - f32(2.0) * dot + pp
    pos_dist = np.sqrt(np.maximum(pos2 + deps2, f32(0.0)))
    losses = np.maximum(pos_dist - max_neg + f32(MARGIN), f32(0.0))
    valid = pos2 != 0
    cnt = f32(valid.sum())
    total = f32(np.sum(losses[valid], dtype=f32))
    return np.asarray(total / cnt, dtype=np.float32)


def run_on_cores(batch, positive, **kwargs):
    nc = _get_nc()
    in_maps = _make_in_maps(batch, positive)
    return bass_utils.run_bass_kernel_spmd(
        nc, in_maps, core_ids=list(range(NCORES)), **kwargs
    )


def kernel(batch, positive):
    batch = np.asarray(batch, dtype=np.float32)
    positive = np.asarray(positive, dtype=np.float32)
    res = run_on_cores(batch, positive)
    return _combine(res.results, positive)



# revision 2
# speedup vs baseline: 2.2178x; 2.2178x over previous
"""Batch triplet loss on 8 TRN2 NeuronCores — fp8 DoubleRow half-Gram, v8.

v7 -> v8: 4.5-panel coverage. The offset-4 block of each core pair is
computed exactly once, split by (row-half, col-half) quarters:
  core c computes rows[0:512] x X_c (slab 8) and rows[512:] x Y_c (slab 9),
  where X_c = cols(c+4)[0:512] if c < 4 else cols(c+4)[512:], Y_c = other.
The partner's quarters arrive via direction-2 (partition-max) like the
offset 1..3 slabs. Saves 1/10 of the Gram matmuls.

Other structure identical to v7: host sq, fold matmul, fp8 DoubleRow,
double-buffered input DMA, gpsimd partition reduce.
"""

import os
from contextlib import ExitStack

import ml_dtypes
import numpy as np

import concourse.bass as bass
import concourse.tile as tile
from concourse import bacc, bass_isa, bass_utils, mybir

N = 8192
D = 1024
NCORES = 8
OWN = N // NCORES       # 1024
KT = D // 128           # 8
JW = 512
NPAN = 5
MOVW = NPAN * OWN       # 5120
NSLAB = MOVW // JW      # 10
NCOL = 9                # m1cols columns per it (slabs 8/9 share col 8)
IT = OWN // 128         # 8
EPS = 1e-6
MARGIN = 0.5

F8 = mybir.dt.float8e4
F16 = mybir.dt.float16
F32 = mybir.dt.float32

_NC = None

D2S = set(range(2, 10))
S_CHUNKS = [[0, 1, 2, 3, 4], [5, 6, 7, 8, 9]]


def _slab_its(s):
    if s == 8:
        return range(0, IT // 2)
    if s == 9:
        return range(IT // 2, IT)
    return range(IT)


def _build_nc():
    REPEAT = int(os.environ.get("KBENCH_REPEAT", "1"))
    HWLOOP = int(os.environ.get("KBENCH_HWLOOP", "0"))  # hw-loop pair count
    nc = bacc.Bacc("TRN2", target_bir_lowering=False, debug=False)
    mov = nc.dram_tensor("mov", [128, KT * MOVW], F8, kind="ExternalInput").ap()
    sqd = nc.dram_tensor("sqd", [1, MOVW], F32, kind="ExternalInput").ap()
    sqid = nc.dram_tensor("sqid", [128, IT], F32, kind="ExternalInput").ap()
    out_m1 = nc.dram_tensor("out_m1", [128, IT], F32, kind="ExternalOutput").ap()
    out_m2 = nc.dram_tensor("out_m2", [1, 8 * JW], F16, kind="ExternalOutput").ap()

    mov_v = mov.rearrange("p (k w) -> p k w", k=KT)  # [128, KT, MOVW]

    with ExitStack() as ctx:
        tc = ctx.enter_context(tile.TileContext(nc))
        const = ctx.enter_context(tc.tile_pool(name="const", bufs=1))
        big = ctx.enter_context(tc.tile_pool(name="big", bufs=1))
        ftp = ctx.enter_context(tc.tile_pool(name="ftp", bufs=4))
        facp = ctx.enter_context(tc.tile_pool(name="facp", bufs=8))
        parp = ctx.enter_context(tc.tile_pool(name="parp", bufs=2))
        resp = ctx.enter_context(tc.tile_pool(name="resp", bufs=1))
        ps_mm = ctx.enter_context(tc.tile_pool(name="ps_mm", bufs=8, space="PSUM"))

        foldw = const.tile([1, 128], F32, tag="foldw", name="foldw")
        nc.vector.memset(foldw[:], -0.5)

        movs2 = [
            big.tile([128, KT * MOVW], F8, tag=f"mv{h}", name=f"movs_all{h}")
            for h in range(2)
        ]
        sqs2 = [resp.tile([1, MOVW], F32, tag=f"sqs{h}", name=f"sqs{h}") for h in range(2)]
        sqi2 = [
            resp.tile([128, IT], F32, tag=f"sqi{h}", name=f"sqi{h}") for h in range(2)
        ]
        m1cols = resp.tile([128, IT * NCOL], F16, tag="m1cols", name="m1cols")
        msb = resp.tile([128, IT], F32, tag="msb", name="msb")
        m2sb = resp.tile([1, 8 * JW], F16, tag="m2sb", name="m2sb")

        def emit_rep(rep_i):
            mv = movs2[rep_i % 2][:].rearrange("p (k w) -> p k w", k=KT)
            sqs = sqs2[rep_i % 2]
            sqi = sqi2[rep_i % 2]
            nc.sync.dma_start(sqs[:], sqd[:])
            nc.sync.dma_start(sqi[:], sqid[:])
            for n in range(NPAN):
                nc.sync.dma_start(
                    mv[:, :, n * OWN : (n + 1) * OWN],
                    mov_v[:, :, n * OWN : (n + 1) * OWN],
                )

            facc = {}
            for s in sorted(D2S):
                f = facp.tile([128, JW], F16, tag="facc", name=f"facc{rep_i}_{s}")
                nc.vector.memset(f[:], -60000.0)
                facc[s] = f

            for it in range(IT):
                for chunk in S_CHUNKS:
                    active = [s for s in chunk if it in _slab_its(s)]
                    psds = {}
                    for s in active:
                        psds[s] = ps_mm.tile(
                            [128, JW], F32, tag="psd", name=f"psd{rep_i}_{it}_{s}"
                        )
                    for t in range(KT // 2):
                        for s in active:
                            nc.tensor.matmul(
                                psds[s][:],
                                mv[:, 2 * t : 2 * t + 2, it * 128 : (it + 1) * 128],
                                mv[:, 2 * t : 2 * t + 2, s * JW : (s + 1) * JW],
                                start=(t == 0),
                                stop=False,
                                perf_mode=mybir.MatmulPerfMode.DoubleRow,
                            )
                    for s in active:
                        nc.tensor.matmul(
                            psds[s][:],
                            foldw[:],
                            sqs[:, s * JW : (s + 1) * JW],
                            start=False,
                            stop=True,
                        )
                    for s in active:
                        ft = ftp.tile([128, JW], F16, tag="ft", name=f"ft{rep_i}_{it}_{s}")
                        nc.scalar.activation(
                            ft[:],
                            psds[s][:],
                            mybir.ActivationFunctionType.Identity,
                            bias=sqi[:, it : it + 1],
                            scale=-2.0,
                        )
                        col = it * NCOL + min(s, 8)
                        nc.vector.reduce_max(
                            m1cols[:, col : col + 1],
                            ft[:],
                            axis=mybir.AxisListType.X,
                            op=mybir.AluOpType.max,
                        )
                        if s in facc:
                            nc.vector.tensor_tensor(
                                facc[s][:], facc[s][:], ft[:], op=mybir.AluOpType.max
                            )

            for s in sorted(D2S):
                par = parp.tile([128, JW], F16, tag="par", name=f"par{rep_i}_{s}")
                nc.gpsimd.partition_all_reduce(
                    par[:], facc[s][:], channels=128, reduce_op=bass_isa.ReduceOp.max
                )
                nc.scalar.copy(m2sb[:, (s - 2) * JW : (s - 1) * JW], par[0:1, :])

            for it in range(IT):
                nc.vector.reduce_max(
                    msb[:, it : it + 1],
                    m1cols[:, it * NCOL : (it + 1) * NCOL],
                    axis=mybir.AxisListType.X,
                    op=mybir.AluOpType.max,
                )

        if HWLOOP:
            with tc.For_i(0, HWLOOP, 1):
                emit_rep(0)
                emit_rep(1)
        else:
            for rep_i in range(REPEAT):
                emit_rep(rep_i)

        nc.gpsimd.dma_start(out_m1[:], msb[:])
        nc.gpsimd.dma_start(out_m2[:], m2sb[:])

    nc.compile()
    return nc


def _get_nc():
    global _NC
    if _NC is None:
        _NC = _build_nc()
    return _NC


def _core_cols(c):
    """Global column indices (batch rows) for core c's MOVW columns."""
    segs = [np.arange(OWN) + ((c + d) % NCORES) * OWN for d in range(4)]
    p4 = ((c + 4) % NCORES) * OWN
    if c < 4:
        segs += [p4 + np.arange(512), p4 + 512 + np.arange(512)]
    else:
        segs += [p4 + 512 + np.arange(512), p4 + np.arange(512)]
    return np.concatenate(segs)


def _make_in_maps(batch, positive):
    bT = np.ascontiguousarray(batch.T)  # [D, N] f32
    b8 = bT.astype(ml_dtypes.float8_e4m3)
    sq_full = np.einsum("ij,ij->j", bT, bT, dtype=np.float32)  # [N] exact
    # d = 128*k + p  ->  [p, k, j_global]
    g = np.ascontiguousarray(b8.reshape(KT, 128, N).transpose(1, 0, 2))
    in_maps = []
    for c in range(NCORES):
        cols = _core_cols(c)
        mov_c = np.ascontiguousarray(g[:, :, cols]).reshape(128, KT * MOVW)
        sqd_c = np.ascontiguousarray(sq_full[cols].reshape(1, MOVW))
        sqi_c = np.ascontiguousarray(
            sq_full[c * OWN : (c + 1) * OWN].reshape(IT, 128).T
        )
        in_maps.append({"mov": mov_c, "sqd": sqd_c, "sqid": sqi_c})
    return in_maps, sq_full


def _combine(results, batch, positive, sq_full):
    f32 = np.float32
    d2max = np.concatenate(
        [results[c]["out_m1"].T.reshape(-1) for c in range(NCORES)]
    ).astype(f32)
    for s in range(2, 10):
        for c in range(NCORES):
            seg = results[c]["out_m2"][0, (s - 2) * JW : (s - 1) * JW].astype(f32)
            if s < 8:
                d_, r_ = s // 2, s % 2
                t = (c + d_) % NCORES
                lo = t * OWN + r_ * JW
            else:
                t = (c + 4) % NCORES
                if s == 8:
                    h = 0 if c < 4 else 1
                else:
                    h = 1 if c < 4 else 0
                lo = t * OWN + h * JW
            d2max[lo : lo + JW] = np.maximum(d2max[lo : lo + JW], seg)

    deps2 = f32(D * EPS * EPS)
    pp = f32(np.sum(positive.astype(f32) ** 2, dtype=f32))
    dotp = batch @ positive  # [N] exact f32
    max_neg = np.sqrt(np.maximum(d2max + deps2, f32(0.0)))
    pos2 = sq_full - f32(2.0) * dotp + pp
    pos_dist = np.sqrt(np.maximum(pos2 + deps2, f32(0.0)))
    losses = np.maximum(pos_dist - max_neg + f32(MARGIN), f32(0.0))
    valid = ~np.all(batch == positive[None, :], axis=1)
    cnt = f32(valid.sum())
    total = f32(np.sum(losses[valid], dtype=f32))
    return np.asarray(total / cnt, dtype=np.float32)


def run_on_cores(batch, positive, **kwargs):
    nc = _get_nc()
    in_maps, sq_full = _make_in_maps(batch, positive)
    res = bass_utils.run_bass_kernel_spmd(
        nc, in_maps, core_ids=list(range(NCORES)), **kwargs
    )
    return res, sq_full


def kernel(batch, positive):
    batch = np.asarray(batch, dtype=np.float32)
    positive = np.asarray(positive, dtype=np.float32)
    res, sq_full = run_on_cores(batch, positive)
    return _combine(res.results, batch, positive, sq_full)
